# revision 16
# baseline (speedup 1.0000x reference)
"""LocallyConnected1D (B=8, L=4096, C=64, K=3, F=64) on 8 TRN2 NeuronCores.

out[b, l, f] = sum_{k,c} x[b, l+k, c] * kernel[l, k, c, f] + bias[l, f]

Strategy (spatial sharding, 512 output positions per core):
  - Pairs of adjacent output positions (2i, 2i+1): stationary tile TE[i]
    (128 x 16) = block-diag(x[2i], x[2i+1]) over (2 phases x 64 channels)
    partitions; streaming operand = per-position fp8 weights.  TO[i] =
    block-diag(x[2i+1], x[2i+2]) serves tap 1.
  - Per pair: tap1 (TO, 64-col matmul) opens the pair, then TE matmuls
    accumulate.  TE[j] serves pair j-1 tap2 AND pair j tap0 whose weight and
    PSUM columns are adjacent -> fused 128-col matmuls (17 MM per group of 8
    pairs instead of 24).  PSUM has_written bits: only the block's first MM
    uses start=True (clears the whole bank); bit=0 -> overwrite handles each
    fresh region after that.
  - All 4 groups of a 32-pair block accumulate into ONE PSUM bank (strip q ->
    partitions 32q..32q+16, tile_position col 32q) so a single full-width
    [128, 512] DVE copy drains the block (vs 4 thin 16-row copies).
  - HBM traffic minimized: weights ship as fp8 e3m4 (x16 host prescale, ~1.4e-2
    max-rel error), x ships as bf16 TE tiles only; TO tiles are built on-chip
    by two partition-shifted DVE copies whose column shift (+8) picks up TE's
    own zero quadrants, so no memsets are needed.  Output returns as bf16.
  - Per block: 2 input DMAs (sync ring), 1 output DMA (scalar ring) -> the
    ~600ns-per-DMA sequencer issue cost stays off the critical path.
"""

import numpy as np
import ml_dtypes

import concourse.bass as bass
import concourse.mybir as mybir
import concourse.tile as tile
from concourse import bacc
from concourse.bass import ds, ts
from concourse.bass_utils import run_bass_kernel_spmd

B, L, C, K, F = 8, 4096, 64, 3, 64
L_OUT = (L - K) + 1  # 4094
N_CORES = 8
P_CORE = 512          # output positions per core (last core: 510 real + 2 pad)
PAIRS = P_CORE // 2   # 256

BLOCKS = [8, 8, 32, 32, 32, 32, 32, 32, 32, 16]   # pairs per block
assert sum(BLOCKS) == PAIRS and all(b % 8 == 0 for b in BLOCKS)
NB = len(BLOCKS)

WSCALE = 16.0  # host-side weight prescale before fp8 e3m4 cast
DT_W = mybir.dt.float8e3
NP_W = ml_dtypes.float8_e3m4
DT_X = mybir.dt.bfloat16
NP_X = ml_dtypes.bfloat16
DT_OUT = mybir.dt.bfloat16


def _w_cols(n):
    return n * K * F


def _te_cols(n):
    return (n + 1) * 16


W_OFF = np.cumsum([0] + [_w_cols(n) for n in BLOCKS]).tolist()
X_OFF = np.cumsum([0] + [_te_cols(n) for n in BLOCKS]).tolist()
W_TOT = W_OFF[-1]
X_TOT = X_OFF[-1]

_CACHE = {}


def _build_body(nc, wpool, xpool, opool, pspool, w_d, x_d, out_d):
    s = 0  # first pair of current block
    for h, n in enumerate(BLOCKS):
        te_c = _te_cols(n)
        to_c = n * 16
        wblk = wpool.tile([128, _w_cols(n)], DT_W, name="wblk", tag="wblk",
                          padded_shape=[128, _w_cols(max(BLOCKS))])
        xblk = xpool.tile([128, te_c + to_c], DT_X, name="xblk", tag="xblk",
                          padded_shape=[128, _te_cols(max(BLOCKS)) +
                                        max(BLOCKS) * 16])
        # Two HWDGE rings (SP=sync, ACT=scalar) each serialize their own
        # transfers; alternate w/x across them so each ring carries ~half the
        # input bytes.  Output DMAs ride the otherwise-idle SWDGE (gpsimd)
        # path so they never head-of-line-block an input ring.
        wq = nc.sync
        xq = nc.sync
        wq.dma_start(wblk[:], w_d[:, ds(W_OFF[h], _w_cols(n))])
        xq.dma_start(xblk[:, ds(0, te_c)], x_d[:, ds(X_OFF[h], te_c)])
        # Build TO tiles from TE tiles: TO[j] = blockdiag(od[j], ev[j+1]).
        # The +8 column shift reads TE's zero quadrants into TO's, so the
        # whole TO region (including zeros) is written.
        nc.vector.tensor_copy(xblk[ds(0, 64), ds(te_c, to_c)],
                              xblk[ds(64, 64), ds(8, to_c)])
        nc.vector.tensor_copy(xblk[ds(64, 64), ds(te_c, to_c)],
                              xblk[ds(0, 64), ds(8, to_c)])

        ngroups = n // 8
        acc = pspool.tile([128, 512], mybir.dt.float32, name="acc", tag="acc")

        def te_ap(i):   # block-diag tile for even-start pair i (global idx)
            return xblk[:, ds((i - s) * 16, 16)]

        def to_ap(i):   # odd-start pair i
            return xblk[:, ds(te_c + (i - s) * 16, 16)]

        def w_ap(jj, k, w=F):
            return wblk[:, ds((jj * K + k) * F, w)]

        # Per group (strip q): TO[0] TE[0] TO[1] TE[1] ... TO[7] TE[7] TE[8].
        # start=True only on the very first MM of the block (clears the whole
        # bank's has_written bits, stale from the pool's previous use).
        for step in range(17):
            for q in range(ngroups):
                g0 = s + q * 8      # first global pair of this group
                jj0 = q * 8         # first in-block pair of this group
                tp = (0, 32 * q)
                j, ph = divmod(step, 2)
                if ph == 0 and j < 8:     # TO[j]: pair j tap1, opens the pair
                    nc.tensor.matmul(acc[ds(32 * q, 16), ts(j, 64)],
                                     to_ap(g0 + j), w_ap(jj0 + j, 1),
                                     start=(step == 0), stop=False,
                                     tile_position=tp, skip_group_check=True)
                elif ph == 1 and j < 8:   # TE[j]
                    if j == 0:            # leading edge: pair 0 tap0 only
                        nc.tensor.matmul(acc[ds(32 * q, 16), ts(0, 64)],
                                         te_ap(g0), w_ap(jj0, 0),
                                         start=False, stop=False,
                                         tile_position=tp,
                                         skip_group_check=True)
                    else:                 # fused: pair j-1 tap2 | pair j tap0
                        nc.tensor.matmul(acc[ds(32 * q, 16),
                                             ds((j - 1) * 64, 128)],
                                         te_ap(g0 + j),
                                         w_ap(jj0 + j - 1, 2, 128),
                                         start=False, stop=False,
                                         tile_position=tp,
                                         skip_group_check=True)
                else:                     # step 16: trailing TE[8], pair7 tap2
                    nc.tensor.matmul(acc[ds(32 * q, 16), ts(7, 64)],
                                     te_ap(g0 + 8), w_ap(jj0 + 7, 2),
                                     start=False, stop=(q == ngroups - 1),
                                     tile_position=tp, skip_group_check=True)
        # One full-width drain: rows 32q..32q+16 hold strip q's outputs,
        # other rows are garbage (shipped; host ignores them).
        rows = 32 * ngroups
        ob = opool.tile([rows, 512], DT_OUT, name="ob", tag="ob",
                        padded_shape=[128, 512])
        nc.vector.tensor_copy(ob[:], acc[ds(0, rows), :])
        nc.scalar.dma_start(out_d[ds(0, rows), ds(h * 512, 512)], ob[:])
        s += n


def _build_nc(n_iters=None):
    """n_iters=None: straight-line kernel (graded path).
    n_iters=N: body wrapped in a HW For_i loop, for timing-slope runs."""
    nc = bacc.Bacc("TRN2", target_bir_lowering=False, debug=False)

    w_d = nc.declare_dram_parameter("wd", [128, W_TOT], DT_W, isOutput=False)
    x_d = nc.declare_dram_parameter("xd", [128, X_TOT], DT_X, isOutput=False)
    # out[p, h*512 + j*64 + f]: p = 32*q + phase*8 + b (rows 32q+16..32q+32
    # garbage), block h strip q covers pairs P0(h) + 8q .. +8q+7.
    out_d = nc.declare_dram_parameter("out", [128, NB * 512], DT_OUT,
                                      isOutput=True)

    with tile.TileContext(nc) as tc:
        with (
            tc.tile_pool(name="wpool", bufs=6) as wpool,
            tc.tile_pool(name="xpool", bufs=6) as xpool,
            tc.tile_pool(name="opool", bufs=4) as opool,
            tc.tile_pool(name="pspool", bufs=4, space=bass.MemorySpace.PSUM) as pspool,
        ):
            if n_iters is None:
                _build_body(nc, wpool, xpool, opool, pspool, w_d, x_d, out_d)
            else:
                with tc.For_i(0, n_iters, 1):
                    _build_body(nc, wpool, xpool, opool, pspool, w_d, x_d,
                                out_d)

    nc.compile()
    return nc


def _prep_inputs(x, kernel):
    """Host-side rearrangement into per-core per-block blobs."""
    xp = np.zeros((B, L + 4, C), np.float32)
    xp[:, :L] = x
    kp = np.zeros((N_CORES * P_CORE, K, C, F), np.float32)
    kp[:L_OUT] = kernel
    in_maps = []
    for m in range(N_CORES):
        l0 = P_CORE * m
        xs = xp[:, l0:l0 + 2 * PAIRS + 2, :]
        ev = xs[:, 0::2].transpose(2, 1, 0)  # (64, 257, 8)  position 2i
        od = xs[:, 1::2].transpose(2, 1, 0)  # (64, 257, 8)  position 2i+1
        # TE[i]: block-diag(x[2i], x[2i+1]) as (128, 16)
        TE = np.zeros((128, PAIRS + 1, 16), np.float32)
        TE[:64, :, 0:8] = ev
        TE[64:, :, 8:16] = od
        W = (kp[l0:l0 + P_CORE]
             .reshape(PAIRS, 2, K, C, F)
             .transpose(1, 3, 0, 2, 4)
             .reshape(128, PAIRS, K, F))  # [pc, pair, k, f]
        wb = np.empty((128, W_TOT), np.float32)
        xb = np.empty((128, X_TOT), np.float32)
        s = 0
        for h, n in enumerate(BLOCKS):
            wb[:, W_OFF[h]:W_OFF[h] + _w_cols(n)] = (
                W[:, s:s + n].reshape(128, _w_cols(n)))
            xb[:, X_OFF[h]:X_OFF[h] + _te_cols(n)] = (
                TE[:, s:s + n + 1].reshape(128, _te_cols(n)))
            s += n
        in_maps.append({
            "wd": (wb * WSCALE).astype(NP_W),
            "xd": xb.astype(NP_X),
        })
    return in_maps


def _unpack_out(res):
    """(128, NB*512) per core -> (B, P_CORE, F).

    res[32q + 16*ph8 ... p = 32q + phase*8 + b, h*512 + j*64 + f];
    l_local = 2*(P0(h) + 8q + j) + phase."""
    r = res.astype(np.float32).reshape(128, NB, 8, 64)  # [p, h, j, f]
    out = np.empty((B, P_CORE, F), np.float32)
    P0 = np.cumsum([0] + BLOCKS).tolist()
    for h, n in enumerate(BLOCKS):
        for q in range(n // 8):
            blk = r[32 * q:32 * q + 16, h].reshape(2, 8, 8, 64)  # ph, b, j, f
            for ph in range(2):
                ls = 2 * (P0[h] + 8 * q) + ph
                # positions ls, ls+2, ..., ls+14  (j = 0..7)
                out[:, ls:ls + 16:2, :] = blk[ph]
    return out


def kernel(x, kernel, bias):
    x = np.asarray(x, dtype=np.float32)
    kern = np.asarray(kernel, dtype=np.float32)
    bias = np.asarray(bias, dtype=np.float32)

    if "nc" not in _CACHE:
        _CACHE["nc"] = _build_nc()
    nc = _CACHE["nc"]

    in_maps = _prep_inputs(x, kern)
    results = run_bass_kernel_spmd(nc, in_maps, list(range(N_CORES))).results

    parts = [_unpack_out(results[m]["out"]) for m in range(N_CORES)]
    out = np.concatenate(parts, axis=1)[:, :L_OUT] * (1.0 / WSCALE)
    return (out + bias[None]).astype(np.float32)


# revision 18
# speedup vs baseline: 1.0602x; 1.0602x over previous
"""LocallyConnected1D (B=8, L=4096, C=64, K=3, F=64) on 8 TRN2 NeuronCores.

out[b, l, f] = sum_{k,c} x[b, l+k, c] * kernel[l, k, c, f] + bias[l, f]

Strategy (spatial sharding, 512 output positions per core):
  - Pairs of adjacent output positions (2i, 2i+1): stationary tile TE[i]
    (128 x 16) = block-diag(x[2i], x[2i+1]) over (2 phases x 64 channels)
    partitions; streaming operand = per-position fp8 weights.  TO[i] =
    block-diag(x[2i+1], x[2i+2]) serves tap 1.
  - Per pair: tap1 (TO, 64-col matmul) opens the pair, then TE matmuls
    accumulate.  TE[j] serves pair j-1 tap2 AND pair j tap0 whose weight and
    PSUM columns are adjacent -> fused 128-col matmuls (17 MM per group of 8
    pairs instead of 24).  start=True on each strip's first matmul clears that
    32-row column-group's has_written bits; bit=0 -> overwrite handles fresh
    regions after that.
  - Groups of 8 pairs map to PE column strips (tile_position col 32q); four
    strips share one PSUM bank (strip q -> partitions 32q..32q+16), drained by
    a single full-width [128, 512] DVE copy.  Blocks larger than 32 pairs span
    several banks, copied into one SBUF staging tile -> ONE output DMA per
    block.
  - HBM traffic and DMA count both minimized (effective DMA BW drops from
    ~317 GB/s at 5-10 DMAs to ~244 GB/s at 20+): weights ship as fp8 e3m4
    (x16 host prescale; ~1.4e-2 max-rel error, PSUM accumulates f32), x ships
    as bf16 TE tiles only; TO tiles are built on-chip by two partition-shifted
    DVE copies whose +8 column shift picks up TE's zero quadrants (no memsets
    needed).  Output returns bf16.  5 blocks -> 10 input DMAs alternating
    across the two HWDGE rings + 5 output DMAs.
"""

import numpy as np
import ml_dtypes

import concourse.bass as bass
import concourse.mybir as mybir
import concourse.tile as tile
from concourse import bacc
from concourse.bass import ds, ts
from concourse.bass_utils import run_bass_kernel_spmd

B, L, C, K, F = 8, 4096, 64, 3, 64
L_OUT = (L - K) + 1  # 4094
N_CORES = 8
P_CORE = 512          # output positions per core (last core: 510 real + 2 pad)
PAIRS = P_CORE // 2   # 256

BLOCKS = [16, 48, 64, 64, 64]   # pairs per block
assert sum(BLOCKS) == PAIRS and all(b % 8 == 0 for b in BLOCKS)
NB = len(BLOCKS)
NGROUPS = [n // 8 for n in BLOCKS]
BG = [(g + 3) // 4 for g in NGROUPS]          # PSUM banks per block
BGOFF = np.cumsum([0] + BG).tolist()          # bank-group offsets
NBG = BGOFF[-1]

WSCALE = 16.0  # host-side weight prescale before fp8 e3m4 cast
DT_W = mybir.dt.float8e3
NP_W = ml_dtypes.float8_e3m4
DT_X = mybir.dt.bfloat16
NP_X = ml_dtypes.bfloat16
DT_OUT = mybir.dt.bfloat16


def _w_cols(n):
    return n * K * F


def _te_cols(n):
    return (n + 1) * 16


W_OFF = np.cumsum([0] + [_w_cols(n) for n in BLOCKS]).tolist()
X_OFF = np.cumsum([0] + [_te_cols(n) for n in BLOCKS]).tolist()
W_TOT = W_OFF[-1]
X_TOT = X_OFF[-1]

_CACHE = {}


def _build_body(nc, wpool, xpool, opool, pspool, w_d, x_d, out_d):
    s = 0  # first pair of current block
    for h, n in enumerate(BLOCKS):
        te_c = _te_cols(n)
        to_c = n * 16
        wblk = wpool.tile([128, _w_cols(n)], DT_W, name="wblk", tag="wblk",
                          padded_shape=[128, _w_cols(max(BLOCKS))])
        xblk = xpool.tile([128, te_c + to_c], DT_X, name="xblk", tag="xblk",
                          padded_shape=[128, _te_cols(max(BLOCKS)) +
                                        max(BLOCKS) * 16])
        wq = nc.sync if h % 2 else nc.scalar
        xq = nc.scalar if h % 2 else nc.sync
        wq.dma_start(wblk[:], w_d[:, ds(W_OFF[h], _w_cols(n))])
        xq.dma_start(xblk[:, ds(0, te_c)], x_d[:, ds(X_OFF[h], te_c)])
        # Build TO tiles from TE tiles: TO[j] = blockdiag(od[j], ev[j+1]).
        # The +8 column shift reads TE's zero quadrants into TO's, so the
        # whole TO region (including its zeros) is written.
        nc.vector.tensor_copy(xblk[ds(0, 64), ds(te_c, to_c)],
                              xblk[ds(64, 64), ds(8, to_c)])
        nc.vector.tensor_copy(xblk[ds(64, 64), ds(te_c, to_c)],
                              xblk[ds(0, 64), ds(8, to_c)])

        def te_ap(i):   # block-diag tile for even-start pair i (global idx)
            return xblk[:, ds((i - s) * 16, 16)]

        def to_ap(i):   # odd-start pair i
            return xblk[:, ds(te_c + (i - s) * 16, 16)]

        def w_ap(jj, k, w=F):
            return wblk[:, ds((jj * K + k) * F, w)]

        ob = opool.tile([128, BG[h] * 512], DT_OUT, name="ob", tag="ob",
                        padded_shape=[128, max(BG) * 512])
        for bg in range(BG[h]):
            nstrip = min(4, NGROUPS[h] - 4 * bg)
            acc = pspool.tile([128, 512], mybir.dt.float32, name="acc",
                              tag="acc")
            # Per strip q: TO[0] TE[0] TO[1] ... TO[7] TE[7] TE[8] (17 MMs).
            for step in range(17):
                for q in range(nstrip):
                    gb = 4 * bg + q     # group within block
                    g0 = s + gb * 8     # first global pair of this group
                    jj0 = gb * 8        # first in-block pair of this group
                    tp = (0, 32 * q)
                    j, ph = divmod(step, 2)
                    if ph == 0 and j < 8:   # TO[j]: pair j tap1, opens pair
                        nc.tensor.matmul(acc[ds(32 * q, 16), ts(j, 64)],
                                         to_ap(g0 + j), w_ap(jj0 + j, 1),
                                         start=(step == 0), stop=False,
                                         tile_position=tp,
                                         skip_group_check=True)
                    elif ph == 1 and j < 8:  # TE[j]
                        if j == 0:          # leading edge: pair 0 tap0 only
                            nc.tensor.matmul(acc[ds(32 * q, 16), ts(0, 64)],
                                             te_ap(g0), w_ap(jj0, 0),
                                             start=False, stop=False,
                                             tile_position=tp,
                                             skip_group_check=True)
                        else:               # fused: pair j-1 tap2|pair j tap0
                            nc.tensor.matmul(acc[ds(32 * q, 16),
                                                 ds((j - 1) * 64, 128)],
                                             te_ap(g0 + j),
                                             w_ap(jj0 + j - 1, 2, 128),
                                             start=False, stop=False,
                                             tile_position=tp,
                                             skip_group_check=True)
                    else:                   # step 16: trailing TE[8]
                        nc.tensor.matmul(acc[ds(32 * q, 16), ts(7, 64)],
                                         te_ap(g0 + 8), w_ap(jj0 + 7, 2),
                                         start=False, stop=(q == nstrip - 1),
                                         tile_position=tp,
                                         skip_group_check=True)
            # Full-width drain of this bank into the block staging tile;
            # rows 32q..32q+16 hold strip q's outputs, rest is garbage.
            nc.vector.tensor_copy(ob[:, ds(bg * 512, 512)], acc[:])
        nc.scalar.dma_start(out_d[:, ds(BGOFF[h] * 512, BG[h] * 512)], ob[:])
        s += n


def _build_nc(n_iters=None):
    """n_iters=None: straight-line kernel (graded path).
    n_iters=N: body wrapped in a HW For_i loop, for timing-slope runs."""
    nc = bacc.Bacc("TRN2", target_bir_lowering=False, debug=False)

    w_d = nc.declare_dram_parameter("wd", [128, W_TOT], DT_W, isOutput=False)
    x_d = nc.declare_dram_parameter("xd", [128, X_TOT], DT_X, isOutput=False)
    # out[p, (BGOFF[h]+bg)*512 + j*64 + f]: p = 32*q + phase*8 + b
    # (rows 32q+16..32q+32 garbage); group gb=4*bg+q covers pairs
    # P0(h)+8*gb .. +7.
    out_d = nc.declare_dram_parameter("out", [128, NBG * 512], DT_OUT,
                                      isOutput=True)

    with tile.TileContext(nc) as tc:
        with (
            tc.tile_pool(name="wpool", bufs=4) as wpool,
            tc.tile_pool(name="xpool", bufs=4) as xpool,
            tc.tile_pool(name="opool", bufs=3) as opool,
            tc.tile_pool(name="pspool", bufs=8, space=bass.MemorySpace.PSUM) as pspool,
        ):
            if n_iters is None:
                _build_body(nc, wpool, xpool, opool, pspool, w_d, x_d, out_d)
            else:
                with tc.For_i(0, n_iters, 1):
                    _build_body(nc, wpool, xpool, opool, pspool, w_d, x_d,
                                out_d)

    nc.compile()
    return nc


def _prep_inputs(x, kernel):
    """Host-side rearrangement into per-core per-block blobs."""
    xp = np.zeros((B, L + 4, C), np.float32)
    xp[:, :L] = x
    kp = np.zeros((N_CORES * P_CORE, K, C, F), np.float32)
    kp[:L_OUT] = kernel
    in_maps = []
    for m in range(N_CORES):
        l0 = P_CORE * m
        xs = xp[:, l0:l0 + 2 * PAIRS + 2, :]
        ev = xs[:, 0::2].transpose(2, 1, 0)  # (64, 257, 8)  position 2i
        od = xs[:, 1::2].transpose(2, 1, 0)  # (64, 257, 8)  position 2i+1
        TE = np.zeros((128, PAIRS + 1, 16), np.float32)
        TE[:64, :, 0:8] = ev
        TE[64:, :, 8:16] = od
        W = (kp[l0:l0 + P_CORE]
             .reshape(PAIRS, 2, K, C, F)
             .transpose(1, 3, 0, 2, 4)
             .reshape(128, PAIRS, K, F))  # [pc, pair, k, f]
        wb = np.empty((128, W_TOT), np.float32)
        xb = np.empty((128, X_TOT), np.float32)
        s = 0
        for h, n in enumerate(BLOCKS):
            wb[:, W_OFF[h]:W_OFF[h] + _w_cols(n)] = (
                W[:, s:s + n].reshape(128, _w_cols(n)))
            xb[:, X_OFF[h]:X_OFF[h] + _te_cols(n)] = (
                TE[:, s:s + n + 1].reshape(128, _te_cols(n)))
            s += n
        in_maps.append({
            "wd": (wb * WSCALE).astype(NP_W),
            "xd": xb.astype(NP_X),
        })
    return in_maps


def _unpack_out(res):
    """(128, NBG*512) per core -> (B, P_CORE, F)."""
    r = res.astype(np.float32).reshape(128, NBG, 8, 64)  # [p, bank, j, f]
    out = np.empty((B, P_CORE, F), np.float32)
    P0 = np.cumsum([0] + BLOCKS).tolist()
    for h, n in enumerate(BLOCKS):
        for bg in range(BG[h]):
            nstrip = min(4, NGROUPS[h] - 4 * bg)
            for q in range(nstrip):
                gb = 4 * bg + q
                blk = r[32 * q:32 * q + 16, BGOFF[h] + bg]
                blk = blk.reshape(2, 8, 8, 64)           # [phase, b, j, f]
                for ph in range(2):
                    ls = 2 * (P0[h] + 8 * gb) + ph
                    out[:, ls:ls + 16:2, :] = blk[ph]
    return out


def kernel(x, kernel, bias):
    x = np.asarray(x, dtype=np.float32)
    kern = np.asarray(kernel, dtype=np.float32)
    bias = np.asarray(bias, dtype=np.float32)

    if "nc" not in _CACHE:
        _CACHE["nc"] = _build_nc()
    nc = _CACHE["nc"]

    in_maps = _prep_inputs(x, kern)
    results = run_bass_kernel_spmd(nc, in_maps, list(range(N_CORES))).results

    parts = [_unpack_out(results[m]["out"]) for m in range(N_CORES)]
    out = np.concatenate(parts, axis=1)[:, :L_OUT] * (1.0 / WSCALE)
    return (out + bias[None]).astype(np.float32)
